# revision 64
# baseline (speedup 1.0000x reference)
"""Trainium2 Bass kernel for ChunkMessagePassing (gnn_message_passing).

Problem: B=2, N=4096, D=512, 3 rounds of causal windowed (W=8) message
passing. Per round:
    A = h @ w1_top + b1 ; Bv = h @ w1_bot         (first MLP layer, factored)
    S[i] = sum_{k=0..8, valid} gelu(A[i] + Bv[i-k])
    U = h @ u1_top + S @ (w2/9 @ u1_bot) + ub1'   (msg 2nd layer fused into
                                                   upd 1st layer on the host;
                                                   b2 folded into ub1')
    new_h = h + gelu(U) @ u2 + ub2 ; h = LN(new_h)

Sharding: 8 cores = B(2) x N-quarters(4). Each core gets 1024 tokens plus a
24-token left halo (3 rounds x window 8), computed redundantly. Zero
cross-core communication. Cores at a sequence start get a zero pad plus a
data-driven edge fixup so all 8 cores run one SPMD program.

Layout: D on partitions (4 tiles of 128), tokens on the free axis. All
state/weights bf16 (PE 1 cyc/row, DVE 2x mode); PSUM accumulates f32.
LN gamma/beta are folded into the round-1/2 weights (two weight sets;
rounds carry x_hat and the residual uses a per-tile diag(gamma) matmul),
so the non-final LN tail is two dt-merged DVE ops. The window stage uses
slot-reordered tmp (even k desc -> slots 0-4, odd k desc -> slots 5-8) so
all 36 shifted adds collapse into 4 wide DVE tensor-tensor instructions
(two token halves) with [dt, k-group, token] 3-free-dim APs (stride-0
broadcast of A, stride +2 over Bv taps; DVE 2x mode tolerates 2-byte
alignment, so no shifted copy is needed). Gelu runs in place per half so
ACT and the DVE tree ping-pong; the tree reduces fully in place and
stage 4 reads S from tmp slot 0 directly. Engine balance: ACT = psum
copies w/ folded biases + gelus + rsqrt (a dummy rsqrt prefetches the
act-table switch); DVE = window adds/tree + Bv copies + x2 + LN tail;
Pool only does the tiny edge fixups (it cannot touch PSUM and is ~4x
slower per column). The round body is chunk-pipelined (3 chunks of ~352
tokens) with next-chunk stage-1 matmuls emitted early to keep the
in-order PE queue fed; the PE sustains only ~1.1-1.2 GHz under the
power throttle, so PE work was cut by fusing away the stage-3 matmul.
"""

import numpy as np
import ml_dtypes

import concourse.bacc as bacc
import concourse.mybir as mybir
from concourse.tile import TileContext
from concourse.bass_utils import run_bass_kernel_spmd
from concourse.ap import AP

f32 = mybir.dt.float32
bf16 = mybir.dt.bfloat16
AF = mybir.ActivationFunctionType
ALU = mybir.AluOpType

B, N, D = 2, 4096, 512
N_ROUNDS = 3
W = 8
W9 = W + 1
NCORES = 8
NLOC = N // 4            # tokens owned per core
HALO = N_ROUNDS * W      # 24
T = NLOC + HALO          # 1048 local tokens incl. halo
DT = 4                   # number of 128-partition d tiles
P = 128
MARG = 8                 # zero margin on the left of Bv buffers
CN = 352                 # max chunk width
CHUNKS = [(0, 352), (352, 352), (704, 344)]
WA = DT * T + 8          # h/A tile width (+8 tail pad for full-CN reads)
BVW = MARG + T + 10      # 1066: margin + T + tail pad
EPS = 1e-5


def build_nc():
    nc = bacc.Bacc("TRN2")

    # ---- DRAM I/O (per-core data supplied via in_maps) ----
    h_in = nc.dram_tensor("h_in", [DT, P, T], bf16, kind="ExternalInput")
    w1t_d = nc.dram_tensor("w1t", [2, DT, P, D], bf16, kind="ExternalInput")
    w1b_d = nc.dram_tensor("w1b", [2, DT, P, D], bf16, kind="ExternalInput")
    u1t_d = nc.dram_tensor("u1t", [2, DT, P, D], bf16, kind="ExternalInput")
    wfu_d = nc.dram_tensor("wfu", [DT, P, D], bf16, kind="ExternalInput")
    u2_d = nc.dram_tensor("u2", [DT, P, D], bf16, kind="ExternalInput")
    b1_d = nc.dram_tensor("b1", [2, P, DT], f32, kind="ExternalInput")
    ub1_d = nc.dram_tensor("ub1f", [2, P, DT], f32, kind="ExternalInput")
    ub2_d = nc.dram_tensor("ub2", [2, P, DT], f32, kind="ExternalInput")
    lng_d = nc.dram_tensor("lng", [P, DT], f32, kind="ExternalInput")
    lnb_d = nc.dram_tensor("lnb", [P, DT], f32, kind="ExternalInput")
    iden_d = nc.dram_tensor("iden", [2, DT, P, P], bf16, kind="ExternalInput")
    ea_d = nc.dram_tensor("edge_a", [P, W], bf16, kind="ExternalInput")
    es_d = nc.dram_tensor("edge_s", [P, W], bf16, kind="ExternalInput")
    hm_d = nc.dram_tensor("hmask", [P, HALO], bf16, kind="ExternalInput")
    out_d = nc.dram_tensor("out", [DT, P, NLOC], f32, kind="ExternalOutput")

    with nc.allow_low_precision("bf16 compute validated against reference"), \
            TileContext(nc) as tc:
        with (
            tc.tile_pool(name="const", bufs=1) as cp,
            tc.tile_pool(name="acts", bufs=1) as ap_,
            tc.tile_pool(name="wp", bufs=2) as wp,
            tc.tile_pool(name="psab", bufs=3, space="PSUM") as psab,
            tc.tile_pool(name="ps", bufs=4, space="PSUM") as ps,
            tc.tile_pool(name="psr", bufs=1, space="PSUM") as psr,
        ):
            # ---- constants into SBUF (index 0: round 0 / raw-h weights;
            # index 1: rounds 1-2 weights with LN gamma/beta folded in) ----
            w1t = [cp.tile([P, DT * D], bf16, tag=f"w1t{i}", name=f"w1t{i}") for i in range(2)]
            w1b = [cp.tile([P, DT * D], bf16, tag=f"w1b{i}", name=f"w1b{i}") for i in range(2)]
            u1t = [cp.tile([P, DT * D], bf16, tag=f"u1t{i}", name=f"u1t{i}") for i in range(2)]
            iden = [cp.tile([P, DT * P], bf16, tag=f"iden{i}", name=f"iden{i}") for i in range(2)]
            wfu = cp.tile([P, DT * D], bf16, tag="wfu")
            u2 = cp.tile([P, DT * D], bf16, tag="u2")
            # round-0 chunk-0 critical loads first (h chunk 0 + stage-1
            # weights), then the rest spread across the DMA-capable queues
            wq = [nc.sync, nc.scalar, nc.gpsimd]
            b1 = [cp.tile([P, DT], f32, tag=f"b1{i}", name=f"b1c{i}") for i in range(2)]
            ub1 = [cp.tile([P, DT], f32, tag=f"ub1{i}", name=f"ub1c{i}") for i in range(2)]
            ub2 = [cp.tile([P, DT], f32, tag=f"ub2{i}", name=f"ub2c{i}") for i in range(2)]
            lng = cp.tile([P, DT], f32, tag="lng")
            lnb = cp.tile([P, DT], f32, tag="lnb")
            edge_a = cp.tile([P, W], bf16, tag="edge_a")
            edge_s = cp.tile([P, W], bf16, tag="edge_s")
            hmask = cp.tile([P, HALO], bf16, tag="hmask")
            for i in range(2):
                nc.scalar.dma_start(out=b1[i][:], in_=b1_d[i])
                nc.scalar.dma_start(out=ub1[i][:], in_=ub1_d[i])
                nc.scalar.dma_start(out=ub2[i][:], in_=ub2_d[i])
            for t_sb, t_d in ((lng, lng_d), (lnb, lnb_d), (edge_a, ea_d),
                              (edge_s, es_d), (hmask, hm_d)):
                nc.scalar.dma_start(out=t_sb[:], in_=t_d[:])
            nc.sync.dma_start(
                out=w1t[0][:].rearrange("p (k d) -> p k d", k=DT),
                in_=w1t_d[0].rearrange("k p d -> p k d"))
            nc.scalar.dma_start(
                out=w1b[0][:].rearrange("p (k d) -> p k d", k=DT),
                in_=w1b_d[0].rearrange("k p d -> p k d"))
            wloads = [(u1t[0], u1t_d, 0), (wfu, wfu_d, None),
                      (u2, u2_d, None), (iden[0], iden_d, 0),
                      (w1t[1], w1t_d, 1), (w1b[1], w1b_d, 1),
                      (u1t[1], u1t_d, 1), (iden[1], iden_d, 1)]
            for j, (t_sb, t_d, idx) in enumerate(wloads):
                src = t_d if idx is None else t_d[idx]
                wq[j % 2].dma_start(
                    out=t_sb[:].rearrange("p (k d) -> p k d", k=DT),
                    in_=src.rearrange("k p d -> p k d"))

            ones_sq = cp.tile([P, P], bf16, tag="ones_sq")
            ones_f = cp.tile([P, 1], bf16, tag="ones_f")
            nc.vector.memset(ones_f[:], 1.0)
            nc.vector.tensor_copy(ones_sq[:], ones_f[:].to_broadcast([P, P]))
            czero = cp.tile([P, 1], f32, tag="czero")
            ceps = cp.tile([P, 1], f32, tag="ceps")
            czb = cp.tile([P, 1], bf16, tag="czb")
            dumo = cp.tile([P, 1], f32, tag="dumo")
            nc.vector.memset(czero[:], 0.0)
            nc.vector.memset(ceps[:], EPS)
            nc.vector.memset(czb[:], 0.0)
            nc.const_aps.aps[(f32, 0.0)] = czero[:]
            nc.const_aps.aps[(f32, EPS)] = ceps[:]
            nc.const_aps.aps[(bf16, 0.0)] = czb[:]

            # ---- activations (persistent) ----
            h0 = ap_.tile([P, WA], bf16, tag="h0")
            h1 = ap_.tile([P, WA], bf16, tag="h1")
            A = ap_.tile([P, WA], bf16, tag="A")       # G aliases into A
            Bv = ap_.tile([P, DT * BVW], bf16, tag="Bv")
            ga8 = ap_.tile([P, DT * W], bf16, tag="ga8")

            nc.vector.memset(Bv[:], 0.0)
            # tail pads (read by full-CN ops, never written)
            nc.vector.memset(h0[:, DT * T:], 0.0)
            nc.vector.memset(h1[:, DT * T:], 0.0)
            nc.vector.memset(A[:, DT * T:], 0.0)

            # round-0 input: chunk 0 goes first on the gpsimd queue, which
            # carries no weight loads, so stage-1 of chunk 0 starts early
            qs0 = [nc.gpsimd, nc.sync]
            for ci, (c0, cn) in enumerate(CHUNKS):
                for dt in range(DT):
                    q = nc.gpsimd if ci == 0 else qs0[(dt + ci) % 2]
                    q.dma_start(
                        out=h0[:, dt * T + c0: dt * T + c0 + cn],
                        in_=h_in[dt, :, c0: c0 + cn])

            def hsl(h, dt, c0, n):
                return h[:, dt * T + c0: dt * T + c0 + n]

            def wtile(w, kt, dt):
                return w[:, kt * D + dt * P: kt * D + dt * P + P]

            def ap3(tile, off, dims):
                t = tile[:]
                return AP(t.tensor, t.offset + off, [list(t.ap[0])] + dims)

            def stage1(hin, ws, c0, cn):
                # A / Bv matmuls + psum->sbuf copies; ws = weight-set index
                for dt in range(DT):
                    pa = psab.tile([P, 512], f32, tag="pab")
                    for kt in range(DT):
                        nc.tensor.matmul(pa[:, :cn], wtile(w1t[ws], kt, dt),
                                         hsl(hin, kt, c0, cn),
                                         start=(kt == 0), stop=(kt == DT - 1))
                    nc.scalar.activation(A[:, dt * T + c0: dt * T + c0 + cn],
                                         pa[:, :cn], AF.Identity,
                                         bias=b1[ws][:, dt: dt + 1])
                    pb = psab.tile([P, 512], f32, tag="pab")
                    for kt in range(DT):
                        nc.tensor.matmul(pb[:, :cn], wtile(w1b[ws], kt, dt),
                                         hsl(hin, kt, c0, cn),
                                         start=(kt == 0), stop=(kt == DT - 1))
                    base = dt * BVW + MARG + c0
                    nc.vector.tensor_copy(Bv[:, base: base + cn], pb[:, :cn])

            hbufs = [h0, h1]
            NCH = len(CHUNKS)

            def emit_stage1(g):
                # stage 1 for global chunk index g (round g//NCH, chunk g%NCH)
                if g >= N_ROUNDS * NCH:
                    return
                rs = g // NCH
                stage1(hbufs[rs % 2], min(rs, 1), *CHUNKS[g % NCH])

            emit_stage1(0)
            emit_stage1(1)
            pending = [None]
            for r in range(N_ROUNDS):
                hin = hbufs[r % 2]
                hout = hbufs[(r + 1) % 2]
                final = (r == N_ROUNDS - 1)

                for ci, (c0, cn) in enumerate(CHUNKS):
                    # ---- edge-fixup prep (chunk 0): gelu of the bare
                    # A taps before the big window gelus occupy ACT
                    if ci == 0:
                        nc.scalar.activation(
                            ga8[:], ap3(A, HALO, [[T, DT], [1, W]]), AF.Gelu)
                        nc.gpsimd.tensor_tensor(
                            ga8[:], ga8[:],
                            ap3(edge_a, 0, [[0, DT], [1, W]]), ALU.mult)

                    # ---- stage 2: windowed gelu-sum -> S, in two token
                    # halves so ACT gelu overlaps DVE adds/tree (ping-pong)
                    tmp = wp.tile([P, DT * W9 * CN], bf16, tag="tmp")
                    HCN = CN // 2
                    HALVES = ((0, HCN), (HCN, CN - HCN))
                    for h0_, hn in HALVES:
                        # even taps k=8,6,4,2,0 -> slots 0-4
                        nc.vector.tensor_tensor(
                            ap3(tmp, h0_, [[W9 * CN, DT], [CN, 5], [1, hn]]),
                            ap3(A, c0 + h0_, [[T, DT], [0, 5], [1, hn]]),
                            ap3(Bv, MARG + c0 + h0_ - 8,
                                [[BVW, DT], [2, 5], [1, hn]]),
                            ALU.add)
                        # odd taps k=7,5,3,1 -> slots 5-8
                        nc.vector.tensor_tensor(
                            ap3(tmp, 5 * CN + h0_,
                                [[W9 * CN, DT], [CN, 4], [1, hn]]),
                            ap3(A, c0 + h0_, [[T, DT], [0, 4], [1, hn]]),
                            ap3(Bv, MARG + c0 + h0_ - 7,
                                [[BVW, DT], [2, 4], [1, hn]]),
                            ALU.add)
                        # gelu in place over all 9 slots of this half
                        nc.scalar.activation(
                            ap3(tmp, h0_, [[W9 * CN, DT], [CN, W9], [1, hn]]),
                            ap3(tmp, h0_, [[W9 * CN, DT], [CN, W9], [1, hn]]),
                            AF.Gelu)
                    for h0_, hn in HALVES:
                        # dt-merged binary-tree reduction, fully in place;
                        # slot 0 ends up holding S and feeds stage 4 directly
                        for wsl, o1 in ((4, 5 * CN), (2, 2 * CN), (1, CN),
                                        (1, 4 * CN)):
                            nc.vector.tensor_tensor(
                                ap3(tmp, h0_, [[W9 * CN, DT], [CN, wsl], [1, hn]]),
                                ap3(tmp, h0_, [[W9 * CN, DT], [CN, wsl], [1, hn]]),
                                ap3(tmp, h0_ + o1,
                                    [[W9 * CN, DT], [CN, wsl], [1, hn]]),
                                ALU.add)

                    # ---- edge fixup (chunk 0; no-op off sequence starts)
                    if ci == 0:
                        sle = ap3(tmp, HALO, [[W9 * CN, DT], [1, W]])
                        nc.vector.tensor_tensor(sle, sle, ga8[:], ALU.subtract)
                        nc.vector.tensor_tensor(
                            sle, sle, ap3(edge_s, 0, [[0, DT], [1, W]]),
                            ALU.mult)

                    # ---- software pipelining: next chunk's stage 1 fills the
                    # PE queue while this chunk's window stage runs on DVE/ACT
                    # (chunks 0-1 of round 0 are pre-emitted to fill warmup)
                    if r * NCH + ci + 1 >= 2:
                        emit_stage1(r * NCH + ci + 1)

                    # deferred LN finish of the previous chunk: its rsqrt
                    # sits behind this chunk's gelus on the ACT queue (no
                    # head-of-line block on the variance chain) and its xn
                    # behind the next stage-1 copies on the DVE queue
                    if pending[0] is not None:
                        pending[0]()
                        pending[0] = None

                    ws = min(r, 1)
                    # ---- stage 4: U = u1t.T@h + wfu.T@S ; G = gelu(U+ub1')
                    # u1t part depends only on hin -> pure PE fill during the
                    # window stage; wfu parts per token half right after each
                    # half's tree finishes
                    pus = [ps.tile([P, 512], f32, tag="pmm", name=f"pu{dt}")
                           for dt in range(DT)]
                    for dt in range(DT):
                        for kt in range(DT):
                            nc.tensor.matmul(
                                pus[dt][:, :cn],
                                wtile(u1t[ws], kt, dt),
                                hsl(hin, kt, c0, cn),
                                start=(kt == 0), stop=False)
                    for dt in range(DT):
                        for kt in range(DT):
                            nc.tensor.matmul(
                                pus[dt][:, :cn],
                                wtile(wfu, kt, dt),
                                tmp[:, kt * W9 * CN: kt * W9 * CN + cn],
                                start=False, stop=(kt == DT - 1))
                    for dt in range(DT):
                        nc.scalar.activation(A[:, dt * T + c0: dt * T + c0 + cn],
                                             pus[dt][:, :cn], AF.Gelu,
                                             bias=ub1[ws][:, dt: dt + 1])

                    # ---- stage 5: V = u2.T@G + (gamma*)h ; hout = V + ub2
                    x2 = wp.tile([P, DT * CN], bf16, tag="x2")
                    pvs = [ps.tile([P, 512], f32, tag="pmm", name=f"pv{dt}")
                           for dt in range(DT)]
                    for dt in range(DT):
                        nc.tensor.matmul(pvs[dt][:, :cn],
                                         iden[ws][:, dt * P: dt * P + P],
                                         hsl(hin, dt, c0, cn),
                                         start=True, stop=False)
                    for dt in range(DT):
                        for kt in range(DT):
                            nc.tensor.matmul(pvs[dt][:, :cn], wtile(u2, kt, dt),
                                             hsl(A, kt, c0, cn),
                                             start=False, stop=(kt == DT - 1))
                    for dt in range(DT):
                        nc.scalar.activation(hsl(hout, dt, c0, cn),
                                             pvs[dt][:, :cn], AF.Identity,
                                             bias=ub2[ws][:, dt: dt + 1])
                        nc.vector.tensor_tensor(
                            x2[:, dt * CN: dt * CN + cn],
                            hsl(hout, dt, c0, cn), hsl(hout, dt, c0, cn),
                            ALU.mult)

                    # ---- stage 6: LN stats via ones-matmul, normalize
                    pr0 = psr.tile([P, 512], f32, tag="prow")
                    pr1 = psr.tile([P, 512], f32, tag="prow")
                    for kt in range(DT):
                        nc.tensor.matmul(pr0[:, :cn], ones_sq[:],
                                         hsl(hout, kt, c0, cn),
                                         start=(kt == 0), stop=(kt == DT - 1))
                    for kt in range(DT):
                        nc.tensor.matmul(pr1[:, :cn], ones_sq[:],
                                         x2[:, kt * CN: kt * CN + cn],
                                         start=(kt == 0), stop=(kt == DT - 1))
                    nmu = wp.tile([P, CN], bf16, tag="nmu")
                    mu2 = wp.tile([P, CN], bf16, tag="mu2")
                    varb = wp.tile([P, CN], f32, tag="varb")
                    if final:
                        rstb = wp.tile([P, CN], f32, tag="rstf")
                    else:
                        rstb = wp.tile([P, CN], bf16, tag="rstb")
                    zb = wp.tile([P, DT * CN], bf16, tag="zb")
                    nc.vector.tensor_scalar_mul(nmu[:, :cn], pr0[:, :cn],
                                                -1.0 / D)
                    # z = hout - mu (start as soon as hout ready; || var path)
                    nc.vector.tensor_tensor(
                        ap3(zb, 0, [[CN, DT], [1, CN]]),
                        ap3(hout, c0, [[T, DT], [1, CN]]),
                        ap3(nmu, 0, [[0, DT], [1, CN]]), ALU.add)
                    nc.vector.tensor_tensor(mu2[:, :cn], nmu[:, :cn],
                                            nmu[:, :cn], ALU.mult)
                    nc.vector.scalar_tensor_tensor(varb[:, :cn], pr1[:, :cn],
                                                   1.0 / D, mu2[:, :cn],
                                                   ALU.mult, ALU.subtract)
                    def ln_finish(varb=varb, rstb=rstb, zb=zb, hout=hout,
                                  c0=c0, cn=cn, ci=ci, final=final):
                        nc.scalar.activation(rstb[:, :cn], varb[:, :cn],
                                             AF.Abs_reciprocal_sqrt, bias=EPS)
                        if not final:
                            # hout <- x_hat = z * rst in place (gamma/beta
                            # live in the next round's folded weights)
                            nc.vector.tensor_tensor(
                                ap3(hout, c0, [[T, DT], [1, cn]]),
                                ap3(zb, 0, [[CN, DT], [1, cn]]),
                                ap3(rstb, 0, [[0, DT], [1, cn]]), ALU.mult)
                            # zero pad margin on sequence-start cores
                            if ci == 0:
                                nc.vector.tensor_tensor(
                                    ap3(hout, 0, [[T, DT], [1, HALO]]),
                                    ap3(hout, 0, [[T, DT], [1, HALO]]),
                                    ap3(hmask, 0, [[0, DT], [1, HALO]]),
                                    ALU.mult)
                        else:
                            xn = wp.tile([P, DT * CN], bf16, tag="xn")
                            nc.vector.tensor_tensor(
                                ap3(xn, 0, [[CN, DT], [1, CN]]),
                                ap3(zb, 0, [[CN, DT], [1, CN]]),
                                ap3(rstb, 0, [[0, DT], [1, CN]]), ALU.mult)
                            fout = wp.tile([P, DT * CN], f32, tag="fout")
                            lo = max(c0, HALO)
                            hi = c0 + cn
                            for dt in range(DT):
                                nc.vector.tensor_scalar(
                                    fout[:, dt * CN: dt * CN + (hi - lo)],
                                    xn[:, dt * CN + (lo - c0): dt * CN + (hi - c0)],
                                    lng[:, dt: dt + 1], lnb[:, dt: dt + 1],
                                    ALU.mult, ALU.add)
                                wq[dt % 3].dma_start(
                                    out=out_d[dt, :, lo - HALO: hi - HALO],
                                    in_=fout[:, dt * CN: dt * CN + (hi - lo)])
                    pending[0] = ln_finish

            if pending[0] is not None:
                pending[0]()
                pending[0] = None

    nc.finalize()
    return nc


_NC_CACHE = {}


def _get_nc():
    if "nc" not in _NC_CACHE:
        _NC_CACHE["nc"] = build_nc()
    return _NC_CACHE["nc"]


def _prep_inputs(chunk_summaries, msg_w1, msg_b1, msg_w2, msg_b2,
                 upd_w1, upd_b1, upd_w2, upd_b2, ln_g, ln_b):
    h = np.asarray(chunk_summaries, np.float32)          # (B, N, D)
    w1 = np.asarray(msg_w1, np.float64)                  # (2D, D)
    w2 = np.asarray(msg_w2, np.float64)                  # (D, D)
    u1 = np.asarray(upd_w1, np.float64)
    u2w = np.asarray(upd_w2, np.float64)
    b2 = np.asarray(msg_b2, np.float64)

    gam = np.asarray(ln_g, np.float64)
    bet = np.asarray(ln_b, np.float64)

    def pack_w(w):
        return np.ascontiguousarray(
            np.asarray(w, np.float64).reshape(DT, P, D).astype(ml_dtypes.bfloat16))

    def pack_w2(wa, wb):
        return np.stack([pack_w(wa), pack_w(wb)])

    def pack_b(b):
        return np.ascontiguousarray(
            np.asarray(b, np.float32).reshape(DT, P).T)

    def pack_b2(ba, bb):
        return np.stack([pack_b(ba), pack_b(bb)])

    wfu = (w2 / 9.0) @ u1[D:]                            # fused msg-l2 -> upd
    ub1f = np.asarray(upd_b1, np.float64) + b2 @ u1[D:]

    # gamma/beta of the LN folded into the round-1/2 weights: the state those
    # rounds carry is x_hat; h = gamma*x_hat + beta is reconstructed by
    # row-scaling the h-consuming weights and folding beta into biases.
    b1f1 = np.asarray(msg_b1, np.float64) + bet @ w1[:D] + bet @ w1[D:]
    ub1f1 = ub1f + bet @ u1[:D]
    ub2f1 = np.asarray(upd_b2, np.float64) + bet
    diag0 = np.stack([np.eye(P) for _ in range(DT)])
    diag1 = np.stack([np.diag(gam[dt * P:(dt + 1) * P]) for dt in range(DT)])

    common = {
        "w1t": pack_w2(w1[:D], gam[:, None] * w1[:D]),
        "w1b": pack_w2(w1[D:], gam[:, None] * w1[D:]),
        "u1t": pack_w2(u1[:D], gam[:, None] * u1[:D]),
        "wfu": pack_w(wfu),
        "u2": pack_w(u2w),
        "b1": pack_b2(msg_b1, b1f1),
        "ub1f": pack_b2(ub1f, ub1f1),
        "ub2": pack_b2(upd_b2, ub2f1),
        "lng": pack_b(ln_g),
        "lnb": pack_b(ln_b),
        "iden": np.stack([diag0, diag1]).astype(ml_dtypes.bfloat16),
    }

    i8 = np.arange(W, dtype=np.float32)
    ea_edge = np.broadcast_to((W - i8), (P, W)).astype(ml_dtypes.bfloat16)
    es_edge = np.broadcast_to((9.0 / (i8 + 1.0)), (P, W)).astype(ml_dtypes.bfloat16)
    ea_mid = np.zeros((P, W), ml_dtypes.bfloat16)
    es_mid = np.ones((P, W), ml_dtypes.bfloat16)
    hm_edge = np.zeros((P, HALO), ml_dtypes.bfloat16)
    hm_mid = np.ones((P, HALO), ml_dtypes.bfloat16)

    in_maps = []
    for core in range(NCORES):
        b = core // 4
        q = core % 4
        n0 = q * NLOC
        if q == 0:
            loc = np.zeros((T, D), np.float32)
            loc[HALO:] = h[b, :NLOC]
            ea, es, hm = ea_edge, es_edge, hm_edge
        else:
            loc = h[b, n0 - HALO: n0 + NLOC]
            ea, es, hm = ea_mid, es_mid, hm_mid
        hloc = np.ascontiguousarray(
            loc.T.reshape(DT, P, T).astype(ml_dtypes.bfloat16))
        m = dict(common)
        m["h_in"] = hloc
        m["edge_a"] = ea
        m["edge_s"] = es
        m["hmask"] = hm
        in_maps.append(m)
    return in_maps


def kernel(**inputs) -> np.ndarray:
    nc = _get_nc()
    in_maps = _prep_inputs(**inputs)
    res = run_bass_kernel_spmd(nc, in_maps, list(range(NCORES)))
    out = np.empty((B, N, D), np.float32)
    for core in range(NCORES):
        b = core // 4
        q = core % 4
        o = res.results[core]["out"]          # (DT, P, NLOC)
        out[b, q * NLOC:(q + 1) * NLOC] = o.reshape(D, NLOC).T
    return out
